# revision 2
# baseline (speedup 1.0000x reference)
"""CSPN accelerate (3x3 per-pixel dynamic filter) on 8 Trainium2 NeuronCores.

out[b,0,h,w] = sum_{di,dj in 0..2} K[b,3*di+dj,h,w] * Xpad[b, h+di-1, w+dj-1]
with the center tap (di=dj=1) taking input0 instead of input.

Sharding: pure data-parallel over batch (16 samples -> 2 per core).

The op is DMA-bandwidth bound (the 9-tap kernel tensor dominates traffic), so
inputs are converted to fp16 on the host and all on-chip traffic is fp16;
only PSUM accumulation is fp32.

The design targets the *real-hardware* DMA cost structure (few large
transfers, large contiguous descriptors, HWDGE-only, no load ever waits on
compute):

  Packed-row layout: partition p in 0..119 holds 4 consecutive image rows
  4p..4p+3 (row-major packed into the free dim), so every kernel / input0 /
  output transfer is 5120B-contiguous per partition:
    - kernel: 3 DMAs per sample of 1.84MB (tap triples, contiguous in DRAM);
      the last sample's last triple is split per-tap (and the final tap in
      half) so the last-arriving bytes gate a fraction of one multiply.
    - x: the host materializes the 6-row halo window with zero borders as
      [120, 6, 642]; ONE 0.92MB DMA per sample with 7704B-contiguous
      descriptors, no memsets. All nine (di,dj) shifted operands are then
      pure free-dim AP slices xt[:, di:di+4, dj:dj+640] -- no shift matmuls.
    - x0 / out: single clean 614KB DMAs ([120, 4, 640] <-> DRAM).
  All tiles are SBUF-resident for both samples (no ring reuse), so the SP
  HWDGE queue issues every load back-to-back and the SDMA engines see one
  uninterrupted ~14MB load stream; stores + half the PSUM drains ride the
  ACT HWDGE ring.

  Compute: DVE does the 9 per-tap fp16 multiplies in packed-row halves
  (2x DVE mode; the strided operand keeps a unit-stride innermost dim); PE
  accumulates the products into fp32 PSUM via identity matmuls (start/stop
  over taps, 8 chunks of [120,320] = one 2KB bank each). Drains are split
  DVE/ACT and stores SP/ACT/ACT so the post-last-byte tail is
  half-product -> 4 matmuls -> one drain -> one 307KB store.
"""

import numpy as np

import concourse.bacc as bacc
import concourse.bass as bass
import concourse.mybir as mybir
import concourse.tile as tile
from concourse.bass_utils import run_bass_kernel_spmd

F16 = mybir.dt.float16
F32 = mybir.dt.float32

BS, KK, H, W = 16, 9, 480, 640
N_CORES = 8
BPC = BS // N_CORES          # samples per core
RJ = 4                       # image rows packed per partition
RP = H // RJ                 # 120 partitions used
WP = W + 2                   # w plus zero pads
WC = 320                     # PSUM chunk columns (fp32, one 2KB bank)
NH = W // WC                 # chunks per packed row (2)


def build_module() -> bass.Bass:
    nc = bacc.Bacc()
    k_ext = nc.declare_dram_parameter("kern", [BPC, KK, RP, RJ, W], F16, isOutput=False)
    x_ext = nc.declare_dram_parameter("x", [BPC, RP, 6, WP], F16, isOutput=False)
    x0_ext = nc.declare_dram_parameter("x0", [BPC, RP, RJ, W], F16, isOutput=False)
    out_ext = nc.declare_dram_parameter("out", [BPC, RP, RJ, W], F16, isOutput=True)

    ident_dram = nc.inline_tensor(np.eye(RP, dtype=np.float16), name="ident")

    with tile.TileContext(nc) as tc:
        with (
            tc.tile_pool(name="consts", bufs=1) as cpool,
            tc.tile_pool(name="kpool", bufs=1) as kpool,
            tc.tile_pool(name="xpool", bufs=1) as xpool,
            tc.tile_pool(name="prodpool", bufs=8) as ppool,
            tc.tile_pool(name="opool", bufs=1) as opool,
            tc.tile_pool(name="psum", bufs=1, space="PSUM") as psumpool,
        ):
            ident = cpool.tile([RP, RP], F16)
            nc.scalar.dma_start(out=ident[:, :], in_=ident_dram[:, :])

            # --- all load DMAs, issued up front (SP HWDGE ring) -------------
            xts, x0ts, kts = [], [], []
            for b in range(BPC):
                xts.append(xpool.tile([RP, 6, WP], F16, tag=f"xt{b}", name=f"xt{b}"))
                x0ts.append(xpool.tile([RP, RJ, W], F16, tag=f"x0t{b}", name=f"x0t{b}"))
                kts.append([
                    kpool.tile([RP, 3, RJ, W], F16, tag=f"kt{b}{g}", name=f"kt{b}{g}")
                    for g in range(3)
                ])
            for b in range(BPC):
                nc.sync.dma_start(out=xts[b][:, :, :], in_=x_ext[b])
                nc.sync.dma_start(out=x0ts[b][:, :, :], in_=x0_ext[b])
                for g in range(3):
                    if b == BPC - 1:
                        # Tail shaping: the last sample loads per-tap so the
                        # DVE product pipeline chases the load stream tap by
                        # tap (1.45us of compute per 1.7us of transfer); the
                        # final tap lands in halves so the last bytes gate
                        # half a multiply.
                        for dj in range(3):
                            if g == 2 and dj == 2:
                                nc.sync.dma_start(out=kts[b][g][:, dj, 0:2, :],
                                                  in_=k_ext[b, 3 * g + dj, :, 0:2, :])
                                nc.sync.dma_start(out=kts[b][g][:, dj, 2:4, :],
                                                  in_=k_ext[b, 3 * g + dj, :, 2:4, :])
                            else:
                                nc.sync.dma_start(out=kts[b][g][:, dj, :, :],
                                                  in_=k_ext[b, 3 * g + dj])
                    else:
                        nc.sync.dma_start(
                            out=kts[b][g][:, :, :, :],
                            in_=k_ext[b, 3 * g:3 * g + 3].transpose([1, 0, 2, 3]),
                        )

            # --- compute + drain + store per sample -------------------------
            psc = [psumpool.tile([RP, WC], F32, tag=f"ps{c}", name=f"ps{c}")
                   for c in range(RJ * NH)]
            for b in range(BPC):
                osb = opool.tile([RP, RJ, W], F16, tag=f"osb{b}", name=f"osb{b}")
                for tap in range(KK):
                    di, dj = tap // 3, tap % 3
                    last_tap = tap == KK - 1
                    prod = ppool.tile([RP, RJ, W], F16, tag="prod", name=f"prod{b}_{tap}")
                    for half in range(2):
                        j0 = 2 * half
                        if di == 1 and dj == 1:
                            in1 = x0ts[b][:, j0:j0 + 2, :]
                        else:
                            in1 = xts[b][:, di + j0:di + j0 + 2, dj:dj + W]
                        nc.vector.tensor_tensor(
                            out=prod[:, j0:j0 + 2, :],
                            in0=kts[b][di][:, dj, j0:j0 + 2, :], in1=in1,
                            op=mybir.AluOpType.mult)
                        # On the closing half of the last tap, finish j3's
                        # chunks first so its drain + store lead the tail.
                        js = (j0 + 1, j0) if (last_tap and half == 1) else (j0, j0 + 1)
                        for j in js:
                            for h in range(NH):
                                nc.tensor.matmul(
                                    out=psc[j * NH + h][:, :],
                                    lhsT=ident[:, :],
                                    rhs=prod[:, j, h * WC:(h + 1) * WC],
                                    start=(tap == 0),
                                    stop=last_tap,
                                )
                # PSUM drains in parallel on both copy-capable engines: ACT
                # takes j0/j1 (whose stop-matmuls finish first, from the
                # first half of the last tap), DVE takes j3/j2 right after
                # its final product. All stores ride the SP ring *behind
                # every load* in program order, so no store ever preempts the
                # load stream; order j3, j0+j1, j2 matches drain completion.
                for j in (0, 1):
                    for h in range(NH):
                        nc.scalar.copy(out=osb[:, j, h * WC:(h + 1) * WC],
                                       in_=psc[j * NH + h][:, :])
                for j in (3, 2):
                    for h in range(NH):
                        nc.vector.tensor_copy(out=osb[:, j, h * WC:(h + 1) * WC],
                                              in_=psc[j * NH + h][:, :])
                if b == BPC - 1:
                    # Tail: the j2 store issues from the ACT ring so the two
                    # final store issues pipeline in parallel rings.
                    nc.sync.dma_start(out=out_ext[b, :, 3, :], in_=osb[:, 3, :])
                    nc.sync.dma_start(out=out_ext[b, :, 0:2, :], in_=osb[:, 0:2, :])
                    nc.scalar.dma_start(out=out_ext[b, :, 2, :], in_=osb[:, 2, :])
                else:
                    nc.sync.dma_start(out=out_ext[b, :, 3, :], in_=osb[:, 3, :])
                    nc.sync.dma_start(out=out_ext[b, :, 0:2, :], in_=osb[:, 0:2, :])
                    nc.sync.dma_start(out=out_ext[b, :, 2, :], in_=osb[:, 2, :])
    nc.finalize()
    return nc


_NC_CACHE = None


def _get_module():
    global _NC_CACHE
    if _NC_CACHE is None:
        _NC_CACHE = build_module()
    return _NC_CACHE


def _pack_x_halo(x: np.ndarray) -> np.ndarray:
    """[BPC,1,H,W] fp16 -> [BPC,RP,6,W+2]: partition p holds image rows
    4p-1..4p+4 with one zero column on each side and zero frame edges."""
    xp = np.zeros((BPC, H + RJ + 2, WP), dtype=np.float16)
    xp[:, 1:H + 1, 1:W + 1] = x[:, 0]
    s0, s1, s2 = xp.strides
    win = np.lib.stride_tricks.as_strided(
        xp, shape=(BPC, RP, 6, WP), strides=(s0, RJ * s1, s1, s2))
    return np.ascontiguousarray(win)


def kernel(**inputs: np.ndarray) -> np.ndarray:
    kern = np.asarray(inputs["kernel"], dtype=np.float16)
    x = np.asarray(inputs["input"], dtype=np.float16)
    x0 = np.asarray(inputs["input0"], dtype=np.float16)
    assert kern.shape == (BS, KK, H, W), kern.shape

    nc = _get_module()
    in_maps = [
        {
            "kern": np.ascontiguousarray(kern[c * BPC:(c + 1) * BPC]).reshape(BPC, KK, RP, RJ, W),
            "x": _pack_x_halo(x[c * BPC:(c + 1) * BPC]),
            "x0": np.ascontiguousarray(x0[c * BPC:(c + 1) * BPC]).reshape(BPC, RP, RJ, W),
        }
        for c in range(N_CORES)
    ]
    res = run_bass_kernel_spmd(nc, in_maps, list(range(N_CORES)))
    out = np.concatenate(
        [res.results[c]["out"].reshape(BPC, 1, H, W) for c in range(N_CORES)], axis=0)
    return out.astype(np.float32)


# revision 3
# speedup vs baseline: 1.0001x; 1.0001x over previous
"""CSPN accelerate (3x3 per-pixel dynamic filter) on 8 Trainium2 NeuronCores.

out[b,0,h,w] = sum_{di,dj in 0..2} K[b,3*di+dj,h,w] * Xpad[b, h+di-1, w+dj-1]
with the center tap (di=dj=1) taking input0 instead of input.

Sharding: pure data-parallel over batch (16 samples -> 2 per core).

The op is DMA-bandwidth bound (the 9-tap kernel tensor dominates traffic), so
inputs are converted to fp16 on the host and all on-chip traffic is fp16;
only PSUM accumulation is fp32.

The design targets the *real-hardware* DMA cost structure (few large
transfers, large contiguous descriptors, HWDGE-only, no load ever waits on
compute):

  Packed-row layout: partition p in 0..119 holds 4 consecutive image rows
  4p..4p+3 (row-major packed into the free dim), so every kernel / input0 /
  output transfer is 5120B-contiguous per partition:
    - kernel: 3 DMAs per sample of 1.84MB (tap triples, contiguous in DRAM);
      the last sample's last triple is split per-tap (and the final tap in
      half) so the last-arriving bytes gate a fraction of one multiply.
    - x: the host materializes the 6-row halo window with zero borders as
      [120, 6, 642]; ONE 0.92MB DMA per sample with 7704B-contiguous
      descriptors, no memsets. All nine (di,dj) shifted operands are then
      pure free-dim AP slices xt[:, di:di+4, dj:dj+640] -- no shift matmuls.
    - x0 / out: single clean 614KB DMAs ([120, 4, 640] <-> DRAM).
  All tiles are SBUF-resident for both samples (no ring reuse), so the SP
  HWDGE queue issues every load back-to-back and the SDMA engines see one
  uninterrupted ~14MB load stream; stores + half the PSUM drains ride the
  ACT HWDGE ring.

  Compute: DVE does the 9 per-tap fp16 multiplies in packed-row halves
  (2x DVE mode; the strided operand keeps a unit-stride innermost dim); PE
  accumulates the products into fp32 PSUM via identity matmuls (start/stop
  over taps, 8 chunks of [120,320] = one 2KB bank each). Drains are split
  DVE/ACT and stores SP/ACT/ACT so the post-last-byte tail is
  half-product -> 4 matmuls -> one drain -> one 307KB store.
"""

import numpy as np

import concourse.bacc as bacc
import concourse.bass as bass
import concourse.mybir as mybir
import concourse.tile as tile
from concourse.bass_utils import run_bass_kernel_spmd

F16 = mybir.dt.float16
F32 = mybir.dt.float32

BS, KK, H, W = 16, 9, 480, 640
N_CORES = 8
BPC = BS // N_CORES          # samples per core
RJ = 4                       # image rows packed per partition
RP = H // RJ                 # 120 partitions used
WP = W + 2                   # w plus zero pads
WC = 320                     # PSUM chunk columns (fp32, one 2KB bank)
NH = W // WC                 # chunks per packed row (2)


def build_module() -> bass.Bass:
    nc = bacc.Bacc()
    k_ext = nc.declare_dram_parameter("kern", [BPC, KK, RP, RJ, W], F16, isOutput=False)
    x_ext = nc.declare_dram_parameter("x", [BPC, RP, 6, WP], F16, isOutput=False)
    x0_ext = nc.declare_dram_parameter("x0", [BPC, RP, RJ, W], F16, isOutput=False)
    out_ext = nc.declare_dram_parameter("out", [BPC, RP, RJ, W], F16, isOutput=True)

    ident_dram = nc.inline_tensor(np.eye(RP, dtype=np.float16), name="ident")

    with tile.TileContext(nc) as tc:
        with (
            tc.tile_pool(name="consts", bufs=1) as cpool,
            tc.tile_pool(name="kpool", bufs=1) as kpool,
            tc.tile_pool(name="xpool", bufs=1) as xpool,
            tc.tile_pool(name="prodpool", bufs=8) as ppool,
            tc.tile_pool(name="opool", bufs=1) as opool,
            tc.tile_pool(name="psum", bufs=1, space="PSUM") as psumpool,
        ):
            ident = cpool.tile([RP, RP], F16)
            nc.scalar.dma_start(out=ident[:, :], in_=ident_dram[:, :])

            # --- all load DMAs, issued up front (SP HWDGE ring) -------------
            xts, x0ts, kts = [], [], []
            for b in range(BPC):
                xts.append(xpool.tile([RP, 6, WP], F16, tag=f"xt{b}", name=f"xt{b}"))
                x0ts.append(xpool.tile([RP, RJ, W], F16, tag=f"x0t{b}", name=f"x0t{b}"))
                kts.append([
                    kpool.tile([RP, 3, RJ, W], F16, tag=f"kt{b}{g}", name=f"kt{b}{g}")
                    for g in range(3)
                ])
            for b in range(BPC):
                nc.sync.dma_start(out=xts[b][:, :, :], in_=x_ext[b])
                nc.sync.dma_start(out=x0ts[b][:, :, :], in_=x0_ext[b])
                for g in range(3):
                    if b == BPC - 1:
                        # Tail shaping: the last sample loads per-tap so the
                        # DVE product pipeline chases the load stream tap by
                        # tap (1.45us of compute per 1.7us of transfer); the
                        # final tap lands in halves so the last bytes gate
                        # half a multiply.
                        for dj in range(3):
                            if g == 2 and dj == 2:
                                nc.sync.dma_start(out=kts[b][g][:, dj, 0:2, :],
                                                  in_=k_ext[b, 3 * g + dj, :, 0:2, :])
                                nc.sync.dma_start(out=kts[b][g][:, dj, 2:4, :],
                                                  in_=k_ext[b, 3 * g + dj, :, 2:4, :])
                            else:
                                nc.sync.dma_start(out=kts[b][g][:, dj, :, :],
                                                  in_=k_ext[b, 3 * g + dj])
                    else:
                        nc.sync.dma_start(
                            out=kts[b][g][:, :, :, :],
                            in_=k_ext[b, 3 * g:3 * g + 3].transpose([1, 0, 2, 3]),
                        )

            # --- compute + drain + store per sample -------------------------
            psc = [psumpool.tile([RP, WC], F32, tag=f"ps{c}", name=f"ps{c}")
                   for c in range(RJ * NH)]
            for b in range(BPC):
                osb = opool.tile([RP, RJ, W], F16, tag=f"osb{b}", name=f"osb{b}")
                for tap in range(KK):
                    di, dj = tap // 3, tap % 3
                    last_tap = tap == KK - 1
                    prod = ppool.tile([RP, RJ, W], F16, tag="prod", name=f"prod{b}_{tap}")
                    for half in range(2):
                        j0 = 2 * half
                        if di == 1 and dj == 1:
                            in1 = x0ts[b][:, j0:j0 + 2, :]
                        else:
                            in1 = xts[b][:, di + j0:di + j0 + 2, dj:dj + W]
                        nc.vector.tensor_tensor(
                            out=prod[:, j0:j0 + 2, :],
                            in0=kts[b][di][:, dj, j0:j0 + 2, :], in1=in1,
                            op=mybir.AluOpType.mult)
                        # On the closing half of the last tap, finish j3's
                        # chunks first so its drain + store lead the tail.
                        js = (j0 + 1, j0) if (last_tap and half == 1) else (j0, j0 + 1)
                        for j in js:
                            for h in range(NH):
                                nc.tensor.matmul(
                                    out=psc[j * NH + h][:, :],
                                    lhsT=ident[:, :],
                                    rhs=prod[:, j, h * WC:(h + 1) * WC],
                                    start=(tap == 0),
                                    stop=last_tap,
                                )
                # PSUM drains run in parallel on both copy-capable engines.
                # Mid-stream samples: all stores ride the SP ring *behind
                # every load* in program order, so no store ever preempts the
                # load stream. Last sample (the exposed tail): j3's chunks
                # stop first (j3-first matmul order above), so ACT drains
                # j3/j2 and issues their stores from its own queue --
                # same-engine ordering drops a cross-engine sem hop -- while
                # DVE drains j0/j1 for the SP-issued j0+j1 store.
                if b == BPC - 1:
                    nc.scalar.copy(out=osb[:, 3, 0:WC], in_=psc[3 * NH][:, :])
                    nc.scalar.copy(out=osb[:, 3, WC:W], in_=psc[3 * NH + 1][:, :])
                    nc.scalar.dma_start(out=out_ext[b, :, 3, :], in_=osb[:, 3, :])
                    nc.scalar.copy(out=osb[:, 2, 0:WC], in_=psc[2 * NH][:, :])
                    nc.scalar.copy(out=osb[:, 2, WC:W], in_=psc[2 * NH + 1][:, :])
                    nc.scalar.dma_start(out=out_ext[b, :, 2, :], in_=osb[:, 2, :])
                    for j in (0, 1):
                        for h in range(NH):
                            nc.vector.tensor_copy(out=osb[:, j, h * WC:(h + 1) * WC],
                                                  in_=psc[j * NH + h][:, :])
                    nc.sync.dma_start(out=out_ext[b, :, 0:2, :], in_=osb[:, 0:2, :])
                else:
                    for j in (0, 1):
                        for h in range(NH):
                            nc.scalar.copy(out=osb[:, j, h * WC:(h + 1) * WC],
                                           in_=psc[j * NH + h][:, :])
                    for j in (3, 2):
                        for h in range(NH):
                            nc.vector.tensor_copy(out=osb[:, j, h * WC:(h + 1) * WC],
                                                  in_=psc[j * NH + h][:, :])
                    nc.sync.dma_start(out=out_ext[b, :, 3, :], in_=osb[:, 3, :])
                    nc.sync.dma_start(out=out_ext[b, :, 0:2, :], in_=osb[:, 0:2, :])
                    nc.sync.dma_start(out=out_ext[b, :, 2, :], in_=osb[:, 2, :])
    nc.finalize()
    return nc


_NC_CACHE = None


def _get_module():
    global _NC_CACHE
    if _NC_CACHE is None:
        _NC_CACHE = build_module()
    return _NC_CACHE


def _pack_x_halo(x: np.ndarray) -> np.ndarray:
    """[BPC,1,H,W] fp16 -> [BPC,RP,6,W+2]: partition p holds image rows
    4p-1..4p+4 with one zero column on each side and zero frame edges."""
    xp = np.zeros((BPC, H + RJ + 2, WP), dtype=np.float16)
    xp[:, 1:H + 1, 1:W + 1] = x[:, 0]
    s0, s1, s2 = xp.strides
    win = np.lib.stride_tricks.as_strided(
        xp, shape=(BPC, RP, 6, WP), strides=(s0, RJ * s1, s1, s2))
    return np.ascontiguousarray(win)


def kernel(**inputs: np.ndarray) -> np.ndarray:
    kern = np.asarray(inputs["kernel"], dtype=np.float16)
    x = np.asarray(inputs["input"], dtype=np.float16)
    x0 = np.asarray(inputs["input0"], dtype=np.float16)
    assert kern.shape == (BS, KK, H, W), kern.shape

    nc = _get_module()
    in_maps = [
        {
            "kern": np.ascontiguousarray(kern[c * BPC:(c + 1) * BPC]).reshape(BPC, KK, RP, RJ, W),
            "x": _pack_x_halo(x[c * BPC:(c + 1) * BPC]),
            "x0": np.ascontiguousarray(x0[c * BPC:(c + 1) * BPC]).reshape(BPC, RP, RJ, W),
        }
        for c in range(N_CORES)
    ]
    res = run_bass_kernel_spmd(nc, in_maps, list(range(N_CORES)))
    out = np.concatenate(
        [res.results[c]["out"].reshape(BPC, 1, H, W) for c in range(N_CORES)], axis=0)
    return out.astype(np.float32)
